# revision 54
# baseline (speedup 1.0000x reference)
"""Multi-head attention (B=2, S=2048, D=1024, H=16, HD=64) on 8 TRN2 cores.

Sharding (hybrid DP/TP, SPMD one-graph):
  core c: batch b = c//4, head-group g = c%4 (heads 4g..4g+3 of batch b).
  QKV Megatron column-split; attention local per (batch, head); att outputs
  AllGather'd per (head, query-half) in the 4-core batch group; O-projection
  column-split on wo; host gather is a pure concat.

All matmul operands bf16 (fp32 PSUM accumulation).
  xT [D, S] bf16 in 4 token-block tiles; qT/kT per j2-slab tiles
  v natural [S, 4 heads, 128] bf16, cols = [v(64) | ones | zeros]
  scoresT [s_k part, s_q free] f32 PSUM; exp on ACT -> bf16
  PV -> psum [128, 512]: rows 0-63 numerator, row 64 denominator.
Head pairs (partitions 0-63 / 64-127) interleave score matmuls so the two
K=64 matmuls run concurrently in distinct PE row groups.  Normalization is
deferred into the next group's score loop so ACT (exp = bottleneck) and PE
never stall on the DVE reciprocal.
"""

import numpy as np
import ml_dtypes

B, S, D = 2, 2048, 1024
H, HD = 16, 64
N_CORES = 8
G = 4
HPC = 4
CW = HPC * HD              # 256
ATT_SCALE = float(HD) ** -0.5
P = 128
NQ = 512

_CACHED_NC = None


def _build():
    import concourse.mybir as mybir
    import concourse.tile as tile
    from concourse import bacc

    f32 = mybir.dt.float32
    bf16 = mybir.dt.bfloat16
    Exp = mybir.ActivationFunctionType.Exp
    add = mybir.AluOpType.add
    mult = mybir.AluOpType.mult

    nc = bacc.Bacc("TRN2", target_bir_lowering=False, debug=False,
                   num_devices=N_CORES)

    KC = D // P           # 8
    SC = S // P           # 16 key chunks
    SB = 4                # token blocks of 512

    # weights / x arrive pre-rearranged from the host so every input DMA is
    # a contiguous per-partition copy (few large descriptors)
    xT = nc.declare_dram_parameter("xT", [SB, P, KC, NQ], bf16,
                                   isOutput=False)
    wq = nc.declare_dram_parameter("wq", [P, KC, CW], bf16, isOutput=False)
    wk = nc.declare_dram_parameter("wk", [P, KC, CW], bf16, isOutput=False)
    wv = nc.declare_dram_parameter("wv", [P, KC, CW], bf16, isOutput=False)
    bq = nc.declare_dram_parameter("bq", [CW], f32, isOutput=False)
    bk = nc.declare_dram_parameter("bk", [CW], f32, isOutput=False)
    bv = nc.declare_dram_parameter("bv", [CW], f32, isOutput=False)
    wo = nc.declare_dram_parameter("wo", [P, KC, CW], bf16, isOutput=False)
    bo = nc.declare_dram_parameter("bo", [CW], f32, isOutput=False)
    # transposed output [outcol, token]; host transposes back (free)
    out = nc.declare_dram_parameter("out", [CW, S], f32, isOutput=True)

    groups = [[0, 1, 2, 3], [4, 5, 6, 7]]

    with tile.TileContext(nc) as tc:
        with (
            tc.tile_pool(name="const", bufs=1) as const,
            tc.tile_pool(name="acts", bufs=1) as acts,
            tc.tile_pool(name="exps", bufs=6) as exps,
            tc.tile_pool(name="attw", bufs=2) as attw,
            tc.tile_pool(name="pvsb", bufs=4) as pvsb,
            tc.tile_pool(name="ostage", bufs=3) as ostage,
            tc.tile_pool(name="dram", bufs=1, space="DRAM") as dram,
        ):
            # ---- input DMAs: small tiles first ------------------------
            bq_sb = const.tile([P, 2], f32, tag="bq")
            bk_sb = const.tile([P, 2], f32, tag="bk")
            nc.sync.dma_start(bq_sb[:], bq.ap().rearrange("(j p) -> p j", p=P))
            nc.sync.dma_start(bk_sb[:], bk.ap().rearrange("(j p) -> p j", p=P))
            wq_sb = const.tile([P, KC, CW], bf16, tag="wq")
            nc.sync.dma_start(wq_sb[:], wq.ap())
            wk_sb = const.tile([P, KC, CW], bf16, tag="wk")
            nc.sync.dma_start(wk_sb[:], wk.ap())
            xts = []
            for sb in range(SB):
                t = const.tile([P, KC, NQ], bf16, tag=f"xt{sb}",
                               name=f"xt{sb}")
                xts.append(t)
                nc.sync.dma_start(t[:], xT[sb])
            wv_sb = const.tile([P, KC, CW], bf16, tag="wv")
            nc.sync.dma_start(wv_sb[:], wv.ap())
            bv_bc = const.tile([P, CW], f32, tag="bv")
            nc.sync.dma_start(bv_bc[:], bv.ap().partition_broadcast(P))
            wo_sb = const.tile([P, KC, CW], bf16, tag="wo")
            nc.sync.dma_start(wo_sb[:], wo.ap())
            bo_T = const.tile([P, 2], f32, tag="bo")
            nc.sync.dma_start(bo_T[:], bo.ap().rearrange("(j p) -> p j", p=P))

            f32r = mybir.dt.float32r
            ones_f = const.tile([P, HD], f32, tag="onesf")
            nc.vector.memset(ones_f[:], 1.0)
            ones_r = const.tile([P, HD], f32r, tag="onesr")
            with nc.allow_low_precision("f32r is fp32 storage"):
                nc.vector.tensor_copy(ones_r[:], ones_f[:])
            # preload the exp table set while DMAs stream
            warm = const.tile([1, 8], bf16, tag="warm")
            warmi = const.tile([1, 8], f32, tag="warmi")
            nc.vector.memset(warmi[:], 0.0)
            nc.scalar.activation(warm[:], warmi[:], Exp, scale=1.0)

            qT = [acts.tile([P, S], bf16, tag=f"qT{j}", name=f"qT{j}")
                  for j in range(2)]
            kT = [acts.tile([P, S], bf16, tag=f"kT{j}", name=f"kT{j}")
                  for j in range(2)]
            v_sb = acts.tile([P, SC, HPC, P], bf16, tag="v")
            nc.vector.memset(v_sb[:, :, :, HD + 1:], 0.0)
            nc.vector.memset(v_sb[:, :, :, HD:HD + 1], 1.0)

            # proj psum pool stays open through attention (side work)
            pp1 = ctx_pp = tc.tile_pool(name="pp1", bufs=2, space="PSUM")
            pp1 = ctx_pp.__enter__()

            def emit_qk_proj(w_sb, b_sb, dst, j, sb, lbl):
                ps = pp1.tile([P, NQ], f32, tag="pq",
                              name=f"pq{lbl}{j}_{sb}")
                for ki in range(KC):
                    nc.tensor.matmul(
                        ps[:], w_sb[:, ki, j * P:(j + 1) * P],
                        xts[sb][:, ki, :],
                        start=(ki == 0), stop=(ki == KC - 1))
                with nc.allow_low_precision("bf16 activations"):
                    nc.vector.tensor_tensor(
                        dst[:, sb * NQ:(sb + 1) * NQ], ps[:],
                        b_sb[:, j:j + 1].to_broadcast((P, NQ)), add)

            def emit_v_proj(si, pair):
                # one head-pair's v columns for token chunk si
                ps = pp1.tile([P, P], f32, tag="pq", name=f"pvv{si}_{pair}")
                for ki in range(KC):
                    nc.tensor.matmul(
                        ps[:],
                        xts[si // 4][:, ki, (si % 4) * P:(si % 4 + 1) * P],
                        wv_sb[:, ki, pair * P:(pair + 1) * P],
                        start=(ki == 0), stop=(ki == KC - 1))
                with nc.allow_low_precision("bf16 activations"):
                    nc.vector.tensor_tensor(
                        v_sb[:, si, 2 * pair:2 * pair + 2, :HD],
                        ps.rearrange("p (h x) -> p h x", x=HD),
                        bv_bc[:, pair * P:(pair + 1) * P]
                        .rearrange("p (h x) -> p h x", x=HD), add)

            # minimal upfront: first quarter's q + all keys for pair 0
            emit_qk_proj(wq_sb, bq_sb, qT[0], 0, 0, "q")
            for sb in range(SB):
                emit_qk_proj(wk_sb, bk_sb, kT[0], 0, sb, "k")

            # ---- attention: groups = (pair, query-quarter) ------------
            GROUP_ORDER = [(0, 0), (0, 1), (0, 2), (0, 3),
                           (1, 0), (1, 1), (1, 2), (1, 3)]
            # side-projection work interleaved into group score loops,
            # keyed (group_idx, mi).  v for a pair is emitted just-in-time
            # inside that pair's first group (PV(mi) needs chunk mi only).
            side = {}
            for si in range(SC):
                side.setdefault((0, si), []).append(
                    lambda si=si: emit_v_proj(si, 0))
                side.setdefault((4, si), []).append(
                    lambda si=si: emit_v_proj(si, 1))
            for q in range(1, SB):            # later quarters' q, pair 0
                side.setdefault((q - 1, 14), []).append(
                    lambda q=q: emit_qk_proj(wq_sb, bq_sb, qT[0], 0, q, "q"))
            for sb in range(SB):              # keys for pair 1, by g4
                gi, mi = (1, 4 + 4 * sb) if sb < 2 else (2, 4 + 4 * (sb - 2))
                side.setdefault((gi, mi), []).append(
                    lambda sb=sb: emit_qk_proj(wk_sb, bk_sb, kT[1], 1, sb,
                                               "k"))
            side.setdefault((3, 6), []).append(
                lambda: emit_qk_proj(wq_sb, bq_sb, qT[1], 1, 0, "q"))
            for q in range(1, SB):            # later quarters' q, pair 1
                side.setdefault((3 + q, 14), []).append(
                    lambda q=q: emit_qk_proj(wq_sb, bq_sb, qT[1], 1, q, "q"))

            agin = {}
            agout = {}
            for pair in range(2):
                for q in range(SB):
                    agin[(pair, q)] = dram.tile(
                        [P, NQ], bf16, tag=f"agi{pair}_{q}",
                        name=f"agi{pair}_{q}")
                    agout[(pair, q)] = dram.tile(
                        [G, P, NQ], bf16, tag=f"ago{pair}_{q}",
                        name=f"ago{pair}_{q}")

            with (
                tc.tile_pool(name="scp", bufs=2, space="PSUM") as scp,
                tc.tile_pool(name="pvp", bufs=2, space="PSUM") as pvp,
            ):
                pending = [None]

                def emit_norm():
                    if pending[0] is None:
                        return
                    pair, q, pvs, rec = pending[0]
                    pending[0] = None
                    # rb tiles live in the sc ring (no extra PSUM banks);
                    # all matmul dsts at partition base 0
                    for x in range(2):
                        rb = scp.tile([HD, NQ], f32, tag="sc",
                                      name=f"rb{pair}_{q}_{x}")
                        nc.tensor.matmul(rb[:],
                                         ones_r[32 * x:32 * x + 1, :],
                                         rec[32 * x:32 * x + 1, :],
                                         start=True, stop=True)
                        at = attw.tile([HD, NQ], bf16, tag="at",
                                       name=f"at{pair}_{q}_{x}")
                        with nc.allow_low_precision("bf16 att"):
                            nc.vector.tensor_tensor(at[:], pvs[x][:], rb[:],
                                                    mult)
                        nc.sync.dma_start(
                            agin[(pair, q)][64 * x:64 * x + HD, :], at[:])
                    nc.gpsimd.collective_compute(
                        "AllGather", mybir.AluOpType.bypass,
                        replica_groups=groups,
                        ins=[agin[(pair, q)].opt()],
                        outs=[agout[(pair, q)].opt()])

                for gi, (pair, q) in enumerate(GROUP_ORDER):
                    pv = [pvp.tile([P, NQ], f32, tag="pv",
                                   name=f"pv{pair}_{q}_{x}")
                          for x in range(2)]  # head A, head B
                    for mi in range(SC):
                        if mi == 5:
                            emit_norm()
                        sc = scp.tile([P, 2 * NQ], f32, tag="sc",
                                      name=f"sc{pair}_{q}_{mi}")
                        for x, off in ((0, 0), (1, HD)):
                            nc.tensor.matmul(
                                sc[:, x * NQ:(x + 1) * NQ],
                                kT[pair][off:off + HD, mi * P:(mi + 1) * P],
                                qT[pair][off:off + HD, q * NQ:(q + 1) * NQ],
                                start=True, stop=True)
                        for fn in side.pop((gi, mi), ()):
                            fn()
                        et = exps.tile([P, 2 * NQ], bf16, tag="exp",
                                       name=f"et{pair}_{q}_{mi}")
                        nc.scalar.activation(et[:], sc[:], Exp,
                                             scale=ATT_SCALE)
                        for x in range(2):
                            nc.tensor.matmul(
                                pv[x][:],
                                v_sb[:, mi, 2 * pair + x, :],
                                et[:, x * NQ:(x + 1) * NQ],
                                start=(mi == 0), stop=(mi == SC - 1))
                    # drain numerators + denominators, free pv banks
                    pvs = [pvsb.tile([HD, NQ], f32, tag="pvs",
                                     name=f"pvs{pair}_{q}_{x}")
                           for x in range(2)]
                    den = attw.tile([33, NQ], f32, tag="den",
                                    name=f"den{pair}_{q}")
                    for x in range(2):
                        nc.vector.tensor_copy(pvs[x][:], pv[x][0:HD, :])
                        nc.vector.tensor_copy(den[32 * x:32 * x + 1, :],
                                              pv[x][HD:HD + 1, :])
                    rec = attw.tile([33, NQ], f32r, tag="rec",
                                    name=f"rec{pair}_{q}")
                    with nc.allow_low_precision("f32r is fp32 storage"):
                        nc.vector.reciprocal(rec[:], den[:])
                    pending[0] = (pair, q, pvs, rec)
                emit_norm()
            ctx_pp.__exit__(None, None, None)

            # ---- O-projection ----------------------------------------
            with tc.tile_pool(name="op", bufs=4, space="PSUM") as op:
                for hs in range(2):
                    # transposed: out[outcol, tok] = wo^T @ att; wo chunk is
                    # the stationary operand so each matmul streams N=512
                    # tokens (64 MMs total instead of 128 N=256 ones)
                    po = [op.tile([P, NQ], f32, tag="po",
                                  name=f"po{hs}_{i}") for i in range(4)]
                    with tc.tile_pool(name=f"attk{hs}", bufs=3) as attk:
                        for kc in range(KC):
                            pair, c = kc // 4, kc % 4
                            atk = attk.tile([P, S // 2], bf16, tag="atk",
                                            name=f"atk{hs}_{kc}")
                            nc.sync.dma_start(
                                atk[:, 0:NQ],
                                agout[(pair, 2 * hs)][c, :, :])
                            nc.sync.dma_start(
                                atk[:, NQ:2 * NQ],
                                agout[(pair, 2 * hs + 1)][c, :, :])
                            for half in range(2):
                                for blk in range(2):
                                    nc.tensor.matmul(
                                        po[2 * half + blk][:],
                                        wo_sb[:, kc,
                                              half * P:(half + 1) * P],
                                        atk[:, blk * NQ:(blk + 1) * NQ],
                                        start=(kc == 0),
                                        stop=(kc == KC - 1))
                        for half in range(2):
                            for blk in range(2):
                                ot = ostage.tile([P, NQ], f32, tag="ot")
                                nc.vector.tensor_tensor(
                                    ot[:], po[2 * half + blk][:],
                                    bo_T[:, hs * 0 + half:half + 1]
                                    .to_broadcast((P, NQ)), add)
                                nc.sync.dma_start(
                                    out[half * P:(half + 1) * P,
                                        hs * 2 * NQ + blk * NQ:
                                        hs * 2 * NQ + (blk + 1) * NQ],
                                    ot[:])

    nc.compile()
    return nc


def _get_nc():
    global _CACHED_NC
    if _CACHED_NC is None:
        _CACHED_NC = _build()
    return _CACHED_NC


# wo row-chunk order matching the on-chip O-proj kc order: kc = (pair, c)
# -> rows = [head 4c+2*pair, head 4c+2*pair+1]
_HEAD_ORDER = [4 * c + 2 * pair + s
               for pair in range(2) for c in range(G) for s in range(2)]


KC_ = D // P
SB_ = 4


def kernel(x, wq, bq, wk, bk, wv, bv, wo, bo):
    from concourse.bass_utils import run_bass_kernel_spmd

    bf = ml_dtypes.bfloat16
    x = np.asarray(x, dtype=np.float32)
    wq = np.asarray(wq, dtype=np.float32)
    wk = np.asarray(wk, dtype=np.float32)
    wv = np.asarray(wv, dtype=np.float32)
    wo = np.asarray(wo, dtype=np.float32)
    bq = np.asarray(bq, dtype=np.float32)
    bk = np.asarray(bk, dtype=np.float32)
    bv = np.asarray(bv, dtype=np.float32)
    bo = np.asarray(bo, dtype=np.float32)

    nc = _get_nc()

    wo_perm = np.ascontiguousarray(
        wo.reshape(H, HD, D)[_HEAD_ORDER].reshape(D, D))

    in_maps = []
    for c in range(N_CORES):
        b, g = c // G, c % G
        cs = slice(g * CW, (g + 1) * CW)
        # pre-rearranged layouts matching SBUF tiles exactly:
        #   w*: (p, ki, m) = w[ki*128+p, m];  xT: (sb, p, ki, n) =
        #   x[b].T[ki*128+p, sb*512+n]
        def wrearr(w):
            return np.ascontiguousarray(
                w.reshape(KC_, P, -1).transpose(1, 0, 2)).astype(bf)

        xb = x[b].T
        in_maps.append({
            "xT": np.ascontiguousarray(
                xb.reshape(KC_, P, SB_, NQ).transpose(2, 1, 0, 3)
            ).astype(bf),
            "wq": wrearr(wq[:, cs]),
            "wk": wrearr(wk[:, cs]),
            "wv": wrearr(wv[:, cs]),
            "bq": np.ascontiguousarray(bq[cs]),
            "bk": np.ascontiguousarray(bk[cs]),
            "bv": np.ascontiguousarray(bv[cs]),
            "wo": wrearr(wo_perm[:, cs]),
            "bo": np.ascontiguousarray(bo[cs]),
        })

    res = run_bass_kernel_spmd(nc, in_maps, core_ids=list(range(N_CORES)))

    full = np.empty((B, S, D), dtype=np.float32)
    for c in range(N_CORES):
        b, g = c // G, c % G
        full[b, :, g * CW:(g + 1) * CW] = res.results[c]["out"].T
    return full
